# revision 1
# baseline (speedup 1.0000x reference)
"""Trainium2 Bass kernel for the binarized 2-layer MLP (eval mode).

Computes, for x [B, 4096] fp32:
    h  = sign(x) @ sign(W1).T + sign(b1)            # [B, 500]
    v  = gamma*(h-mean)*rsqrt(var+eps) + beta
    s2 = sign(clip(v, -1, 1)) = sign(v)
    out = s2 @ sign(W2).T + sign(b2)                # [B, 12]

Strategy: pure data parallel over 8 NeuronCores (2048 rows each).  All the
BN / bias / sign algebra on the small tensors is folded on the host into a
per-feature threshold + sign-folded weights, so the device only computes
sign(x), two integer-exact matmuls and one thresholded Sign.

Device pipeline per core (mode "v2"):
  - SWDGE DMA with fp32->bf16 cast loads x row-tiles [128, 4096]
  - PE 128x128 transpose (identity matmul, bf16) -> PSUM
  - binarize PSUM->SBUF fp8: ACT Sign (+-1) for k-chunks < 16, DVE
    (is_ge 0) - 0.5 (+-0.5) for k-chunks >= 16 (weights pre-scaled 2x there)
  - layer-1 matmul in fp8 with perf_mode=DoubleRow (K=256 per instruction)
  - ACT Sign(h + thr) per-partition threshold -> s2 fp8
  - layer-2 fp8 matmul (12 outputs) + bias -> strided DMA of out.T to HBM
All matmul contributions are +-1 accumulated in fp32 PSUM: bit-exact.
"""

from contextlib import ExitStack

import ml_dtypes
import numpy as np

import concourse.bass as bass
import concourse.tile as tile
from concourse import bacc, mybir
from concourse.bass_utils import run_bass_kernel_spmd

N_CORES = 8
B, D, H, C = 16384, 4096, 500, 12
ROWS = B // N_CORES  # rows of x per core
BN_EPS = 1e-5

P = 128          # partitions
KC = D // P      # 32 k-chunks of 128 features
KK = KC // 2     # 16 DoubleRow k-chunks of 256 features
MT = 4           # m-chunks of the 500 hidden features
MSZ = H // MT    # 125
HP = 512         # padded H in the DoubleRow weight layout
NCHUNK = 512     # rows processed per chunk
ACT_KC = KC // 2  # k-chunks [0, ACT_KC) binarized on ACT (+-1), rest DVE

F32 = mybir.dt.float32
BF16 = mybir.dt.bfloat16
FP8 = mybir.dt.float8e4
NP_BF16 = ml_dtypes.bfloat16
NP_FP8 = ml_dtypes.float8_e4m3

MODE = "v2"
V2_LIKE = ("v2", "v3")


def build(rows=ROWS, mode=MODE, reps=1):
    """Build the per-core Bass program for `rows` rows of x.

    reps > 1 repeats the whole compute (including the HBM reads of x) —
    used only for device-time measurement via marginal cost."""
    assert rows % NCHUNK == 0
    n_chunks = rows // NCHUNK
    tiles_per_chunk = NCHUNK // P  # 4

    nc = bacc.Bacc("TRN2", target_bir_lowering=False, debug=False,
                   num_devices=N_CORES)

    x = nc.dram_tensor("x", [rows, D], F32, kind="ExternalInput").ap()
    if mode in ("v2", "v2b", "v3"):
        w1t = nc.dram_tensor("w1t", [P, KK, 2, HP], FP8,
                             kind="ExternalInput").ap()
        w2t = nc.dram_tensor("w2t", [MSZ, MT * C], FP8,
                             kind="ExternalInput").ap()
        ident = nc.dram_tensor("ident", [P, P],
                               BF16 if mode in ("v2", "v3") else F32,
                               kind="ExternalInput").ap()
    else:
        w1t = nc.dram_tensor("w1t", [D, H], BF16, kind="ExternalInput").ap()
        w2t = nc.dram_tensor("w2t", [MSZ, MT * C], BF16,
                             kind="ExternalInput").ap()
        ident = nc.dram_tensor("ident", [P, P], F32,
                               kind="ExternalInput").ap()
    thr = nc.dram_tensor("thr", [MSZ, MT], F32, kind="ExternalInput").ap()
    bias2 = nc.dram_tensor("bias2", [C, 1], F32, kind="ExternalInput").ap()
    out = nc.dram_tensor("out", [rows, C], F32, kind="ExternalOutput").ap()

    with tile.TileContext(nc) as tc, ExitStack() as ctx:
        consts = ctx.enter_context(tc.tile_pool(name="consts", bufs=1))
        xpool = ctx.enter_context(tc.tile_pool(name="x", bufs=6))
        xtpool = ctx.enter_context(tc.tile_pool(name="xt", bufs=2))
        s2pool = ctx.enter_context(tc.tile_pool(name="s2", bufs=8))
        opool = ctx.enter_context(tc.tile_pool(name="o", bufs=2))
        ps_tr = ctx.enter_context(tc.tile_pool(name="ps_tr", bufs=5, space="PSUM"))
        ps_h = ctx.enter_context(tc.tile_pool(name="ps_h", bufs=2, space="PSUM"))
        ps_o = ctx.enter_context(tc.tile_pool(name="ps_o", bufs=1, space="PSUM"))

        # one-time constant loads (weight loads deferred below so the x
        # stream owns the SDMA engines from t=0)
        if mode in ("v2", "v2b", "v3"):
            w1t_sb = consts.tile([P, KK, 2, HP], FP8)
            w2t_sb = consts.tile([MSZ, MT * C], FP8)
        else:
            w1t_sb = consts.tile([P, KC, H], BF16)
            w2t_sb = consts.tile([MSZ, MT * C], BF16)
        thr_sb = consts.tile([MSZ, MT], F32)
        nc.sync.dma_start(thr_sb[:], thr[:])
        bias2_sb = consts.tile([C, 1], F32)
        nc.sync.dma_start(bias2_sb[:], bias2[:])
        ident_sb = consts.tile([P, P], BF16 if mode in ("v2", "v3") else F32)
        nc.sync.dma_start(ident_sb[:], ident[:])

        x_dt = BF16 if mode in ("v2", "v3") else F32
        s_dt = FP8 if mode in ("v2", "v2b", "v3") else BF16
        KG = 8 if mode in ("v2", "v3") else 4  # k-chunks per transpose-PSUM tile

        def load_weights():
            if mode in ("v2", "v2b", "v3"):
                nc.sync.dma_start(w1t_sb[:], w1t[:])
            else:
                nc.sync.dma_start(w1t_sb[:], w1t.rearrange("(kc p) h -> p kc h", p=P))
            nc.sync.dma_start(w2t_sb[:], w2t[:])

        for chi, ch in enumerate([c for _ in range(reps) for c in range(n_chunks)]):
            # transpose + binarize into xT, consuming one x row-tile at a time
            if mode in ("v2", "v2b", "v3"):
                xT = xtpool.tile([P, KK, 2, NCHUNK], FP8, tag="xT")
            else:
                xT = xtpool.tile([P, KC, NCHUNK], BF16, tag="xT")
            for t in range(tiles_per_chunk):
                xt_ = xpool.tile([P, D], x_dt, tag="x")
                row0 = (ch * tiles_per_chunk + t) * P
                if mode in ("v2", "v3"):
                    # SWDGE DMA with fp32 -> bf16 cast
                    nc.gpsimd.dma_start(xt_[:], x[row0:row0 + P, :])
                else:
                    nc.sync.dma_start(xt_[:], x[row0:row0 + P, :])
                if chi == 0 and t == 0:
                    load_weights()
                for kg in range(KC // KG):
                    pst = ps_tr.tile([P, KG * P], x_dt, tag="tr")
                    for j in range(KG):
                        kc = KG * kg + j
                        nc.tensor.matmul(
                            pst[:, j * P:(j + 1) * P],
                            xt_[:, kc * P:(kc + 1) * P],
                            ident_sb[:],
                            is_transpose=True,
                            skip_group_check=True,
                        )
                    if mode in ("v2", "v2b", "v3"):
                        kk0 = KG * kg // 2
                        dst = xT[:, kk0:kk0 + KG // 2, :, t * P:(t + 1) * P]
                    else:
                        dst = xT[:, KG * kg:KG * (kg + 1), t * P:(t + 1) * P]
                    if KG * kg < ACT_KC:
                        nc.scalar.activation(
                            dst, pst[:], mybir.ActivationFunctionType.Sign)
                    else:
                        nc.vector.tensor_scalar(
                            dst, pst[:], 0.0, 0.5,
                            mybir.AluOpType.is_ge, mybir.AluOpType.subtract)

            # layer 1: h_mm[mc] accumulated over k
            s2_tiles = []
            for mc in range(MT):
                psh = ps_h.tile([MSZ, NCHUNK], F32, tag="h")
                if mode in ("v2", "v2b", "v3"):
                    for kk in range(KK):
                        nc.tensor.matmul(
                            psh[:],
                            w1t_sb[:, kk, :, mc * MSZ:(mc + 1) * MSZ],
                            xT[:, kk, :, :],
                            start=(kk == 0),
                            stop=(kk == KK - 1),
                            perf_mode=mybir.MatmulPerfMode.DoubleRow,
                        )
                else:
                    for kc in range(KC):
                        nc.tensor.matmul(
                            psh[:],
                            w1t_sb[:, kc, mc * MSZ:(mc + 1) * MSZ],
                            xT[:, kc, :],
                            start=(kc == 0),
                            stop=(kc == KC - 1),
                        )
                s2 = s2pool.tile([MSZ, NCHUNK], s_dt, tag="s2")
                nc.scalar.activation(
                    s2[:], psh[:], mybir.ActivationFunctionType.Sign,
                    bias=thr_sb[:, mc:mc + 1], scale=1.0)
                s2_tiles.append(s2)

            # layer 2: out.T [12, 512]
            pso = ps_o.tile([C, NCHUNK], F32, tag="o")
            for mc in range(MT):
                nc.tensor.matmul(
                    pso[:],
                    w2t_sb[:, mc * C:(mc + 1) * C],
                    s2_tiles[mc][:],
                    start=(mc == 0),
                    stop=(mc == MT - 1),
                )
            if mode == "v3":
                # pad to 32 partitions, 32x32 DVE transpose, then a DMA with
                # 48-byte contiguous runs (vs 4-byte runs of the naive
                # rearranged-AP DMA, which HW executes ~an order of magnitude
                # slower than the descriptor model suggests)
                o_sb = opool.tile([32, NCHUNK], F32, tag="osb")
                nc.vector.memset(o_sb[:], 0.0)
                nc.scalar.activation(
                    o_sb[:C, :], pso[:], mybir.ActivationFunctionType.Identity,
                    bias=bias2_sb[:, 0:1], scale=1.0)
                z_sb = opool.tile([32, NCHUNK], F32, tag="zsb")
                nc.vector.transpose(z_sb[:], o_sb[:])
                # z_sb[p, 32*b + j] = out[ch*NCHUNK + 32*b + p, j]
                z_src = z_sb[:].rearrange("p (b j) -> p b j", j=32)[:, :, :C]
                dst = out[ch * NCHUNK:(ch + 1) * NCHUNK, :].rearrange(
                    "(b p) c -> p b c", p=32)
                nc.sync.dma_start(dst, z_src)
            else:
                o_sb = opool.tile([C, NCHUNK], F32, tag="osb")
                nc.scalar.activation(
                    o_sb[:], pso[:], mybir.ActivationFunctionType.Identity,
                    bias=bias2_sb[:, 0:1], scale=1.0)
                nc.sync.dma_start(
                    out[ch * NCHUNK:(ch + 1) * NCHUNK, :].rearrange("n c -> c n"),
                    o_sb[:])

    nc.finalize()
    return nc


def _sgn(v):
    return np.where(v >= 0, 1.0, -1.0)


def prep_consts(W1, b1, gamma, beta, running_mean, running_var, W2, b2,
                mode=MODE):
    """Fold all small-tensor algebra into device constants (float64 host math)."""
    f8 = np.float64
    sW1 = _sgn(W1.astype(f8))                       # [H, D]
    scale_k = np.ones(D, f8)
    scale_k[ACT_KC * P:] = 2.0                      # DVE chunks encode x as +-0.5
    w1_scaled = (sW1 * scale_k[None, :]).T          # [D, H]

    inv = 1.0 / np.sqrt(running_var.astype(f8) + BN_EPS)
    a = gamma.astype(f8) * inv
    c = beta.astype(f8) - gamma.astype(f8) * running_mean.astype(f8) * inv
    sb1 = _sgn(b1.astype(f8))
    safe_a = np.where(a == 0, 1.0, a)
    thr_feat = np.where(a != 0, sb1 + c / safe_a, 0.0)   # [H]
    sgn_a = np.where(a > 0, 1.0, np.where(a < 0, -1.0, 0.0))

    sW2 = _sgn(W2.astype(f8))                       # [C, H]
    W2f = sW2 * sgn_a[None, :]                      # zero where a == 0
    const_feat = (a == 0)
    bias2_np = _sgn(b2.astype(f8)) + (sW2[:, const_feat]
                                      * _sgn(c[const_feat])[None, :]).sum(axis=1)

    if mode in ("v2", "v2b", "v3"):
        # DoubleRow layout: w1dr[p, kk, i, m] = w1_scaled[256*kk + 128*i + p, m]
        w1dr = np.zeros((P, KK, 2, HP), f8)
        w1dr[:, :, :, :H] = w1_scaled.reshape(KK, 2, P, H).transpose(2, 0, 1, 3)
        w1t_np = w1dr.astype(NP_FP8)
        w_dt = NP_FP8
        ident_np = (np.eye(P, dtype=NP_BF16) if mode in ("v2", "v3")
                    else np.eye(P, dtype=np.float32))
    else:
        w1t_np = np.ascontiguousarray(w1_scaled).astype(NP_BF16)  # [D, H]
        w_dt = NP_BF16
        ident_np = np.eye(P, dtype=np.float32)

    thr_np = np.ascontiguousarray(
        thr_feat.reshape(MT, MSZ).T).astype(np.float32)          # [125, 4]
    w2t_np = np.ascontiguousarray(
        W2f.T.reshape(MT, MSZ, C).transpose(1, 0, 2).reshape(MSZ, MT * C)
    ).astype(w_dt)                                                # [125, 4*12]
    bias2_np = bias2_np.reshape(C, 1).astype(np.float32)
    return dict(w1t=w1t_np, thr=thr_np, w2t=w2t_np, bias2=bias2_np,
                ident=ident_np)


_cached = {}


def _get_nc(rows=ROWS, mode=MODE, reps=1):
    key = (rows, mode, reps)
    if key not in _cached:
        _cached[key] = build(rows, mode, reps)
    return _cached[key]


def kernel(x, W1, b1, gamma, beta, running_mean, running_var, W2, b2):
    x = np.ascontiguousarray(np.asarray(x, dtype=np.float32))
    consts = prep_consts(np.asarray(W1), np.asarray(b1), np.asarray(gamma),
                         np.asarray(beta), np.asarray(running_mean),
                         np.asarray(running_var), np.asarray(W2),
                         np.asarray(b2))
    nc = _get_nc()
    in_maps = []
    for i in range(N_CORES):
        m = {"x": np.ascontiguousarray(x[i * ROWS:(i + 1) * ROWS])}
        m.update(consts)
        in_maps.append(m)
    res = run_bass_kernel_spmd(nc, in_maps, core_ids=list(range(N_CORES)),
                               trace=False)
    out = np.concatenate([r["out"] for r in res.results], axis=0)
    kernel.last_results = res
    return out

